# revision 4
# baseline (speedup 1.0000x reference)
"""CMoE (hash-routed top-1 MoE + RWKV time-shift mix) on 8 TRN2 NeuronCores.

Strategy: expert-parallel. Core e owns expert e's weights. The hash
routing is a pure function of token_ids, so the dispatch/combine
(all-to-all in the sharding hint) is realized at input-sharding time: the
host computes each expert's kept-token list, gathers those tokens'
x/x-prev rows into feature-major per-core buffers, and scatters the
per-core outputs back into [B, T, m]. Everything with FLOPs — the
time-shift mix, all three matmuls, relu^2, sigmoid, and the r*kv
product — runs on-device.

Per-core device kernel (one expert, capacity 2048 tokens, 4 chunks of
512): activations live feature-major ([m-tile planes, tokens]) so every
matmul uses the natural weight layout as the stationary operand and
chains without any transposes. Matmuls run bf16 (inputs are rounded
on-chip by the DVE ops that produce them), accumulation fp32 in PSUM.
DMA is spread over three rings (sync: activations/outputs, scalar: Wk
stream, gpsimd: Wv/Wr) with explicit pacing deps so big transfers never
starve the critical path; the next chunk's loads + mix are
software-pipelined into the current chunk's second phase.
"""
from contextlib import ExitStack

import numpy as np
import ml_dtypes

import concourse.bacc as bacc
import concourse.tile as tile
import concourse.mybir as mybir
from concourse.tile import add_dep_helper
from concourse.bass_utils import run_bass_kernel_spmd

F32 = mybir.dt.float32
BF16 = mybir.dt.bfloat16
AF = mybir.ActivationFunctionType
AL = mybir.AluOpType

HASH_PRIME = 5099
B, T, M, F, E = 8, 2048, 1024, 2048, 8
S = B * T
CAP = max(4, -(-S // E))   # 2048
CT = 512
NM = M // 128              # 8
NF = F // 128              # 16
NC = CAP // CT             # 4


def _build():
    nc = bacc.Bacc(None)
    xg_d = nc.dram_tensor("xg", [NC, 128, NM, CT], BF16, kind="ExternalInput")
    xp_d = nc.dram_tensor("xp", [NC, 128, NM, CT], BF16, kind="ExternalInput")
    maak_d = nc.dram_tensor("maak", [128, NM], F32, kind="ExternalInput")
    maar_d = nc.dram_tensor("maar", [128, NM], F32, kind="ExternalInput")
    wk_d = nc.dram_tensor("wk", [NF, 128, NM, 128], BF16, kind="ExternalInput")
    wv_d = nc.dram_tensor("wv", [NM, 128, NF, 128], BF16, kind="ExternalInput")
    wr_d = nc.dram_tensor("wr", [NM, 128, NM, 128], BF16, kind="ExternalInput")
    out_d = nc.dram_tensor("outT", [NC, 128, NM, CT], F32, kind="ExternalOutput")

    with tile.TileContext(nc) as tc, ExitStack() as ctx:
        maa_p = ctx.enter_context(tc.tile_pool(name="maa", bufs=1))
        wr_p = ctx.enter_context(tc.tile_pool(name="wr", bufs=1))
        wk_p = ctx.enter_context(tc.tile_pool(name="wk", bufs=6))
        wv_p = ctx.enter_context(tc.tile_pool(name="wv", bufs=3))
        act_p = ctx.enter_context(tc.tile_pool(name="act", bufs=1))
        h_p = ctx.enter_context(tc.tile_pool(name="h", bufs=2))
        scr_p = ctx.enter_context(tc.tile_pool(name="scr", bufs=2))
        out_p = ctx.enter_context(tc.tile_pool(name="out", bufs=3))
        hps_p = ctx.enter_context(tc.tile_pool(name="hps", bufs=3, space="PSUM"))
        kvps_p = ctx.enter_context(tc.tile_pool(name="kvps", bufs=2, space="PSUM"))
        rps_p = ctx.enter_context(tc.tile_pool(name="rps", bufs=2, space="PSUM"))

        maak = maa_p.tile([128, NM], F32, name="maak_sb")
        maar = maa_p.tile([128, NM], F32, name="maar_sb")
        nc.sync.dma_start(maak[:], maak_d[:])
        nc.sync.dma_start(maar[:], maar_d[:])
        wr_sb = wr_p.tile([128, NM, NM, 128], BF16, name="wr_sb")

        def load_act_plane(ct, a, xg, xp):
            d1 = nc.sync.dma_start(xg[:, a, :], xg_d[ct][:, a, :])
            d2 = nc.sync.dma_start(xp[:, a, :], xp_d[ct][:, a, :])
            return [d1, d2]

        def mix_plane(xg, xp, xk, xr, a):
            dx = scr_p.tile([128, CT], F32, name="dx_sb", tag="dx")
            nc.vector.tensor_sub(dx[:], xp[:, a, :], xg[:, a, :])
            nc.vector.scalar_tensor_tensor(
                xk[:, a, :], dx[:], maak[:, a:a + 1], xg[:, a, :],
                op0=AL.mult, op1=AL.add)
            nc.vector.scalar_tensor_tensor(
                xr[:, a, :], dx[:], maar[:, a:a + 1], xg[:, a, :],
                op0=AL.mult, op1=AL.add)

        def new_act_tiles():
            xg = act_p.tile([128, NM, CT], BF16, name="xg_sb", tag="xg", bufs=1)
            xp = act_p.tile([128, NM, CT], BF16, name="xp_sb", tag="xp", bufs=1)
            xk = act_p.tile([128, NM, CT], BF16, name="xk_sb", tag="xk", bufs=2)
            xr = act_p.tile([128, NM, CT], BF16, name="xr_sb", tag="xr", bufs=2)
            return xg, xp, xk, xr

        # PE warmup: the HAM clock-gate holds the PE at 1.2GHz until it has
        # seen ~3.4us of sustained activity. Chunk 0's matmuls arrive in a
        # DMA-paced drip and would otherwise all run cold; burn ~8us of
        # zero matmuls first so real work starts at 2.4GHz.
        wpad = maa_p.tile([128, CT], BF16, name="warm_sb")
        nc.gpsimd.memset(wpad[:], 0)
        wps = hps_p.tile([128, CT], F32, name="warm_ps", tag="hps")
        for _ in range(36):
            nc.tensor.matmul(wps[:], wpad[:, :128], wpad[:],
                             start=True, stop=True, skip_group_check=True)

        # chunk 0 prologue: per-plane load + mix, so mm1 starts after the
        # first plane lands rather than after the full 4MB.
        xg, xp, xk, xr = new_act_tiles()
        for a in range(NM):
            load_act_plane(0, a, xg, xp)
            mix_plane(xg, xp, xk, xr, a)

        for ct in range(NC):
            # h^T[ft] = relu(Wk_ft^T @ xk)^2  -> h_sb bf16
            h_sb = h_p.tile([128, NF, CT], BF16, name="h_sb", tag="h")
            relus = []
            for ft in range(NF):
                wk_sb = wk_p.tile([128, NM, 128], BF16, name="wk_sb", tag="wk")
                nc.scalar.dma_start(wk_sb[:], wk_d[ft])
                hps = hps_p.tile([128, CT], F32, name="hps", tag="hps")
                for a in range(NM):
                    nc.tensor.matmul(hps[:], wk_sb[:, a, :], xk[:, a, :],
                                     start=(a == 0), stop=(a == NM - 1))
                hr = scr_p.tile([128, CT], F32, name="hr_sb", tag="hr")
                relus.append(nc.scalar.activation(hr[:], hps[:], AF.Relu))
                nc.vector.tensor_mul(h_sb[:, ft, :], hr[:], hr[:])

            if ct == 0:
                # wr's first consumer is the mt-loop below; 8 pieces paced
                # across mm1 so piece mt lands just before its first use.
                for wmt in range(NM):
                    wr_dma = nc.gpsimd.dma_start(wr_sb[:, wmt, :, :], wr_d[wmt])
                    add_dep_helper(wr_dma.ins, relus[min(2 + wmt, NF - 1)].ins,
                                   True, "pace wr piece behind mm1 progress")

            # software-pipelined next chunk: plane loads gated on mm1
            # progress; mix planes woven through the mt-loop below. All
            # planes must be in flight early enough that the woven mix never
            # blocks the in-order DVE stream mid-chunk.
            if ct + 1 < NC:
                xg_n, xp_n, xk_n, xr_n = new_act_tiles()
                for a in range(NM):
                    for d in load_act_plane(ct + 1, a, xg_n, xp_n):
                        add_dep_helper(d.ins, relus[min(a, NF - 1)].ins,
                                       True, "pace next-chunk act loads")
            else:
                xg_n = xp_n = xk_n = xr_n = None

            # kv^T[mt] = Wv_mt^T @ h ; r^T[mt] = sigmoid(Wr_mt^T @ xr)
            # out^T[mt] = r^T * kv^T
            for mt in range(NM):
                wv_sb = wv_p.tile([128, NF, 128], BF16, name="wv_sb", tag="wv")
                wv_dma = nc.gpsimd.dma_start(wv_sb[:], wv_d[mt])
                if ct == 0 and mt < 3:
                    add_dep_helper(wv_dma.ins, relus[7 + 2 * mt].ins, True,
                                   "pace early wv loads behind mm1 progress")
                kvps = kvps_p.tile([128, CT], F32, name="kvps", tag="kvps")
                for ftk in range(NF):
                    nc.tensor.matmul(kvps[:], wv_sb[:, ftk, :], h_sb[:, ftk, :],
                                     start=(ftk == 0), stop=(ftk == NF - 1))
                rps = rps_p.tile([128, CT], F32, name="rps", tag="rps")
                for kt in range(NM):
                    nc.tensor.matmul(rps[:], wr_sb[:, mt, kt, :], xr[:, kt, :],
                                     start=(kt == 0), stop=(kt == NM - 1))
                rsb = scr_p.tile([128, CT], F32, name="r_sb", tag="r")
                nc.scalar.activation(rsb[:], rps[:], AF.Sigmoid)
                ot = out_p.tile([128, CT], F32, name="ot_sb", tag="ot")
                nc.vector.tensor_mul(ot[:], rsb[:], kvps[:])
                nc.sync.dma_start(out_d[ct][:, mt, :], ot[:])
                if xg_n is not None:
                    mix_plane(xg_n, xp_n, xk_n, xr_n, mt)

            xg, xp, xk, xr = xg_n, xp_n, xk_n, xr_n

    nc.finalize()
    return nc


_NC_CACHE = None
last_run = None  # BassKernelResults of the most recent kernel() call


def _get_nc():
    global _NC_CACHE
    if _NC_CACHE is None:
        _NC_CACHE = _build()
    return _NC_CACHE


def _route(token_ids):
    """Reference-equivalent hash routing with capacity drop. Returns each
    expert's kept-token flat indices (arrival order) and their time-shift
    predecessor indices into [x_flat; shift_state]."""
    tok = token_ids.reshape(S).astype(np.int64)
    e_idx = (tok * HASH_PRIME) % E
    order = np.argsort(e_idx, kind="stable")
    counts = np.bincount(e_idx, minlength=E)
    starts = np.zeros(E, np.int64)
    starts[1:] = np.cumsum(counts)[:-1]
    prev = np.arange(S) - 1
    prev[np.arange(B) * T] = S + np.arange(B)   # row 0 reads shift_state
    idx, pidx = [], []
    for e in range(E):
        ke = order[starts[e]: starts[e] + min(counts[e], CAP)]
        idx.append(ke)
        pidx.append(prev[ke])
    return idx, pidx


def kernel(x, token_ids, shift_state, time_maa_k, time_maa_r, Wk, Wv, Wr):
    global last_run
    x = np.ascontiguousarray(np.asarray(x, dtype=np.float32))
    token_ids = np.asarray(token_ids)
    shift_state = np.asarray(shift_state, dtype=np.float32)
    Wk = np.asarray(Wk, dtype=np.float32)
    Wv = np.asarray(Wv, dtype=np.float32)
    Wr = np.asarray(Wr, dtype=np.float32)

    idx, pidx = _route(token_ids)

    x_flat = x.reshape(S, M)
    xcatT = np.ascontiguousarray(
        np.concatenate([x_flat, shift_state], axis=0).T)   # [M, S+B]
    mk = np.ascontiguousarray(
        np.asarray(time_maa_k, dtype=np.float32).reshape(M).reshape(8, 128).T)
    mr = np.ascontiguousarray(
        np.asarray(time_maa_r, dtype=np.float32).reshape(M).reshape(8, 128).T)

    in_maps = []
    for e in range(E):
        n_e = len(idx[e])
        xg = np.zeros((M, CAP), np.float32)
        xp = np.zeros((M, CAP), np.float32)
        xg[:, :n_e] = xcatT[:, idx[e]]
        xp[:, :n_e] = xcatT[:, pidx[e]]
        # [M, CAP] -> [NC, 128, NM, CT] chunk-major with m = a*128 + p, so a
        # chunk plane's load is fully contiguous per SBUF partition.
        xg_dev = np.ascontiguousarray(
            xg.reshape(NM, 128, NC, CT).transpose(2, 1, 0, 3)
        ).astype(ml_dtypes.bfloat16)
        xp_dev = np.ascontiguousarray(
            xp.reshape(NM, 128, NC, CT).transpose(2, 1, 0, 3)
        ).astype(ml_dtypes.bfloat16)
        wk_dev = np.ascontiguousarray(
            Wk[e].reshape(NM, 128, NF, 128).transpose(2, 1, 0, 3)
        ).astype(ml_dtypes.bfloat16)
        wv_dev = np.ascontiguousarray(
            Wv[e].reshape(NF, 128, NM, 128).transpose(2, 1, 0, 3)
        ).astype(ml_dtypes.bfloat16)
        wr_dev = np.ascontiguousarray(
            Wr[e].reshape(NM, 128, NM, 128).transpose(2, 1, 0, 3)
        ).astype(ml_dtypes.bfloat16)
        in_maps.append({
            "xg": xg_dev, "xp": xp_dev, "maak": mk, "maar": mr,
            "wk": wk_dev, "wv": wv_dev, "wr": wr_dev,
        })

    import os
    trace = os.environ.get("MOE_KERNEL_TRACE", "0") == "1"
    kw = {}
    if trace:
        kw = {"trace": True, "trace_cores": [0]}
    res = run_bass_kernel_spmd(_get_nc(), in_maps,
                               core_ids=list(range(E)), **kw)
    last_run = res

    out_flat = np.zeros((S, M), np.float32)
    for e in range(E):
        outT = res.results[e]["outT"]            # [NC, 128, NM, CT]
        rows = outT.transpose(0, 3, 2, 1).reshape(CAP, M)
        out_flat[idx[e]] = rows[: len(idx[e])]
    out = out_flat.reshape(B, T, M)
    new_shift_state = np.ascontiguousarray(x[:, -1])
    return out, new_shift_state


# revision 5
# speedup vs baseline: 1.0030x; 1.0030x over previous
"""CMoE (hash-routed top-1 MoE + RWKV time-shift mix) on 8 TRN2 NeuronCores.

Strategy: expert-parallel. Core e owns expert e's weights. The hash
routing is a pure function of token_ids, so the dispatch/combine
(all-to-all in the sharding hint) is realized at input-sharding time: the
host computes each expert's kept-token list, gathers those tokens'
x/x-prev rows into feature-major per-core buffers, and scatters the
per-core outputs back into [B, T, m]. Everything with FLOPs — the
time-shift mix, all three matmuls, relu^2, sigmoid, and the r*kv
product — runs on-device.

Per-core device kernel (one expert, capacity 2048 tokens, 4 chunks of
512): activations live feature-major ([m-tile planes, tokens]) so every
matmul uses the natural weight layout as the stationary operand and
chains without any transposes. Matmuls run bf16 (inputs are rounded
on-chip by the DVE ops that produce them), accumulation fp32 in PSUM.
DMA is spread over three rings (sync: activations/outputs, scalar: Wk
stream, gpsimd: Wv/Wr) with explicit pacing deps so big transfers never
starve the critical path; the next chunk's loads + mix are
software-pipelined into the current chunk's second phase.
"""
from contextlib import ExitStack

import numpy as np
import ml_dtypes

import concourse.bacc as bacc
import concourse.tile as tile
import concourse.mybir as mybir
from concourse.tile import add_dep_helper
from concourse.bass_utils import run_bass_kernel_spmd

F32 = mybir.dt.float32
BF16 = mybir.dt.bfloat16
AF = mybir.ActivationFunctionType
AL = mybir.AluOpType

HASH_PRIME = 5099
B, T, M, F, E = 8, 2048, 1024, 2048, 8
S = B * T
CAP = max(4, -(-S // E))   # 2048
CT = 512
NM = M // 128              # 8
NF = F // 128              # 16
NC = CAP // CT             # 4


def _build():
    nc = bacc.Bacc(None)
    xg_d = nc.dram_tensor("xg", [NC, 128, NM, CT], BF16, kind="ExternalInput")
    xp_d = nc.dram_tensor("xp", [NC, 128, NM, CT], BF16, kind="ExternalInput")
    maak_d = nc.dram_tensor("maak", [128, NM], F32, kind="ExternalInput")
    maar_d = nc.dram_tensor("maar", [128, NM], F32, kind="ExternalInput")
    wk_d = nc.dram_tensor("wk", [NF, 128, NM, 128], BF16, kind="ExternalInput")
    wv_d = nc.dram_tensor("wv", [NM, 128, NF, 128], BF16, kind="ExternalInput")
    wr_d = nc.dram_tensor("wr", [NM, 128, NM, 128], BF16, kind="ExternalInput")
    out_d = nc.dram_tensor("outT", [NC, 128, NM, CT], F32, kind="ExternalOutput")

    with tile.TileContext(nc) as tc, ExitStack() as ctx:
        maa_p = ctx.enter_context(tc.tile_pool(name="maa", bufs=1))
        wr_p = ctx.enter_context(tc.tile_pool(name="wr", bufs=1))
        wk_p = ctx.enter_context(tc.tile_pool(name="wk", bufs=6))
        wv_p = ctx.enter_context(tc.tile_pool(name="wv", bufs=3))
        act_p = ctx.enter_context(tc.tile_pool(name="act", bufs=1))
        h_p = ctx.enter_context(tc.tile_pool(name="h", bufs=2))
        scr_p = ctx.enter_context(tc.tile_pool(name="scr", bufs=2))
        out_p = ctx.enter_context(tc.tile_pool(name="out", bufs=3))
        hps_p = ctx.enter_context(tc.tile_pool(name="hps", bufs=3, space="PSUM"))
        kvps_p = ctx.enter_context(tc.tile_pool(name="kvps", bufs=2, space="PSUM"))
        rps_p = ctx.enter_context(tc.tile_pool(name="rps", bufs=2, space="PSUM"))

        maak = maa_p.tile([128, NM], F32, name="maak_sb")
        maar = maa_p.tile([128, NM], F32, name="maar_sb")
        nc.sync.dma_start(maak[:], maak_d[:])
        nc.sync.dma_start(maar[:], maar_d[:])
        wr_sb = wr_p.tile([128, NM, NM, 128], BF16, name="wr_sb")

        def load_act_plane(ct, a, xg, xp):
            # alternate planes across the two HWDGE rings so arrivals
            # overlap instead of serializing on one FIFO
            eng = nc.sync if a % 2 == 0 else nc.scalar
            d1 = eng.dma_start(xg[:, a, :], xg_d[ct][:, a, :])
            d2 = eng.dma_start(xp[:, a, :], xp_d[ct][:, a, :])
            return [d1, d2]

        def mix_plane(xg, xp, xk, xr, a):
            dx = scr_p.tile([128, CT], F32, name="dx_sb", tag="dx")
            nc.vector.tensor_sub(dx[:], xp[:, a, :], xg[:, a, :])
            nc.vector.scalar_tensor_tensor(
                xk[:, a, :], dx[:], maak[:, a:a + 1], xg[:, a, :],
                op0=AL.mult, op1=AL.add)
            nc.vector.scalar_tensor_tensor(
                xr[:, a, :], dx[:], maar[:, a:a + 1], xg[:, a, :],
                op0=AL.mult, op1=AL.add)

        def new_act_tiles():
            xg = act_p.tile([128, NM, CT], BF16, name="xg_sb", tag="xg", bufs=1)
            xp = act_p.tile([128, NM, CT], BF16, name="xp_sb", tag="xp", bufs=1)
            xk = act_p.tile([128, NM, CT], BF16, name="xk_sb", tag="xk", bufs=2)
            xr = act_p.tile([128, NM, CT], BF16, name="xr_sb", tag="xr", bufs=2)
            return xg, xp, xk, xr

        # PE warmup: the HAM clock-gate holds the PE at 1.2GHz until it has
        # seen ~3.4us of sustained activity. Chunk 0's matmuls arrive in a
        # DMA-paced drip and would otherwise all run cold; burn ~8us of
        # zero matmuls first so real work starts at 2.4GHz.
        wpad = maa_p.tile([128, CT], BF16, name="warm_sb")
        nc.gpsimd.memset(wpad[:], 0)
        wps = hps_p.tile([128, CT], F32, name="warm_ps", tag="hps")
        for _ in range(36):
            nc.tensor.matmul(wps[:], wpad[:, :128], wpad[:],
                             start=True, stop=True, skip_group_check=True)

        # chunk 0 prologue: per-plane load + mix, so mm1 starts after the
        # first plane lands rather than after the full 4MB.
        xg, xp, xk, xr = new_act_tiles()
        for a in range(NM):
            load_act_plane(0, a, xg, xp)
            mix_plane(xg, xp, xk, xr, a)

        for ct in range(NC):
            # h^T[ft] = relu(Wk_ft^T @ xk)^2  -> h_sb bf16
            h_sb = h_p.tile([128, NF, CT], BF16, name="h_sb", tag="h")
            relus = []
            for ft in range(NF):
                wk_sb = wk_p.tile([128, NM, 128], BF16, name="wk_sb", tag="wk")
                nc.scalar.dma_start(wk_sb[:], wk_d[ft])
                hps = hps_p.tile([128, CT], F32, name="hps", tag="hps")
                for a in range(NM):
                    nc.tensor.matmul(hps[:], wk_sb[:, a, :], xk[:, a, :],
                                     start=(a == 0), stop=(a == NM - 1))
                hr = scr_p.tile([128, CT], F32, name="hr_sb", tag="hr")
                relus.append(nc.scalar.activation(hr[:], hps[:], AF.Relu))
                nc.vector.tensor_mul(h_sb[:, ft, :], hr[:], hr[:])

            if ct == 0:
                # wr's first consumer is the mt-loop below; 8 pieces paced
                # across mm1 so piece mt lands just before its first use.
                for wmt in range(NM):
                    wr_dma = nc.gpsimd.dma_start(wr_sb[:, wmt, :, :], wr_d[wmt])
                    add_dep_helper(wr_dma.ins, relus[min(2 + wmt, NF - 1)].ins,
                                   True, "pace wr piece behind mm1 progress")

            # software-pipelined next chunk: plane loads gated on mm1
            # progress; mix planes woven through the mt-loop below. All
            # planes must be in flight early enough that the woven mix never
            # blocks the in-order DVE stream mid-chunk.
            if ct + 1 < NC:
                xg_n, xp_n, xk_n, xr_n = new_act_tiles()
                for a in range(NM):
                    for d in load_act_plane(ct + 1, a, xg_n, xp_n):
                        add_dep_helper(d.ins, relus[min(a, NF - 1)].ins,
                                       True, "pace next-chunk act loads")
            else:
                xg_n = xp_n = xk_n = xr_n = None

            # kv^T[mt] = Wv_mt^T @ h ; r^T[mt] = sigmoid(Wr_mt^T @ xr)
            # out^T[mt] = r^T * kv^T
            for mt in range(NM):
                wv_sb = wv_p.tile([128, NF, 128], BF16, name="wv_sb", tag="wv")
                wv_dma = nc.gpsimd.dma_start(wv_sb[:], wv_d[mt])
                if ct == 0 and mt < 3:
                    add_dep_helper(wv_dma.ins, relus[7 + 2 * mt].ins, True,
                                   "pace early wv loads behind mm1 progress")
                kvps = kvps_p.tile([128, CT], F32, name="kvps", tag="kvps")
                for ftk in range(NF):
                    nc.tensor.matmul(kvps[:], wv_sb[:, ftk, :], h_sb[:, ftk, :],
                                     start=(ftk == 0), stop=(ftk == NF - 1))
                rps = rps_p.tile([128, CT], F32, name="rps", tag="rps")
                for kt in range(NM):
                    nc.tensor.matmul(rps[:], wr_sb[:, mt, kt, :], xr[:, kt, :],
                                     start=(kt == 0), stop=(kt == NM - 1))
                rsb = scr_p.tile([128, CT], F32, name="r_sb", tag="r")
                nc.scalar.activation(rsb[:], rps[:], AF.Sigmoid)
                ot = out_p.tile([128, CT], F32, name="ot_sb", tag="ot")
                nc.vector.tensor_mul(ot[:], rsb[:], kvps[:])
                nc.sync.dma_start(out_d[ct][:, mt, :], ot[:])
                if xg_n is not None:
                    mix_plane(xg_n, xp_n, xk_n, xr_n, mt)

            xg, xp, xk, xr = xg_n, xp_n, xk_n, xr_n

    nc.finalize()
    return nc


_NC_CACHE = None
last_run = None  # BassKernelResults of the most recent kernel() call


def _get_nc():
    global _NC_CACHE
    if _NC_CACHE is None:
        _NC_CACHE = _build()
    return _NC_CACHE


def _route(token_ids):
    """Reference-equivalent hash routing with capacity drop. Returns each
    expert's kept-token flat indices (arrival order) and their time-shift
    predecessor indices into [x_flat; shift_state]."""
    tok = token_ids.reshape(S).astype(np.int64)
    e_idx = (tok * HASH_PRIME) % E
    order = np.argsort(e_idx, kind="stable")
    counts = np.bincount(e_idx, minlength=E)
    starts = np.zeros(E, np.int64)
    starts[1:] = np.cumsum(counts)[:-1]
    prev = np.arange(S) - 1
    prev[np.arange(B) * T] = S + np.arange(B)   # row 0 reads shift_state
    idx, pidx = [], []
    for e in range(E):
        ke = order[starts[e]: starts[e] + min(counts[e], CAP)]
        idx.append(ke)
        pidx.append(prev[ke])
    return idx, pidx


def kernel(x, token_ids, shift_state, time_maa_k, time_maa_r, Wk, Wv, Wr):
    global last_run
    x = np.ascontiguousarray(np.asarray(x, dtype=np.float32))
    token_ids = np.asarray(token_ids)
    shift_state = np.asarray(shift_state, dtype=np.float32)
    Wk = np.asarray(Wk, dtype=np.float32)
    Wv = np.asarray(Wv, dtype=np.float32)
    Wr = np.asarray(Wr, dtype=np.float32)

    idx, pidx = _route(token_ids)

    x_flat = x.reshape(S, M)
    xcatT = np.ascontiguousarray(
        np.concatenate([x_flat, shift_state], axis=0).T)   # [M, S+B]
    mk = np.ascontiguousarray(
        np.asarray(time_maa_k, dtype=np.float32).reshape(M).reshape(8, 128).T)
    mr = np.ascontiguousarray(
        np.asarray(time_maa_r, dtype=np.float32).reshape(M).reshape(8, 128).T)

    in_maps = []
    for e in range(E):
        n_e = len(idx[e])
        xg = np.zeros((M, CAP), np.float32)
        xp = np.zeros((M, CAP), np.float32)
        xg[:, :n_e] = xcatT[:, idx[e]]
        xp[:, :n_e] = xcatT[:, pidx[e]]
        # [M, CAP] -> [NC, 128, NM, CT] chunk-major with m = a*128 + p, so a
        # chunk plane's load is fully contiguous per SBUF partition.
        xg_dev = np.ascontiguousarray(
            xg.reshape(NM, 128, NC, CT).transpose(2, 1, 0, 3)
        ).astype(ml_dtypes.bfloat16)
        xp_dev = np.ascontiguousarray(
            xp.reshape(NM, 128, NC, CT).transpose(2, 1, 0, 3)
        ).astype(ml_dtypes.bfloat16)
        wk_dev = np.ascontiguousarray(
            Wk[e].reshape(NM, 128, NF, 128).transpose(2, 1, 0, 3)
        ).astype(ml_dtypes.bfloat16)
        wv_dev = np.ascontiguousarray(
            Wv[e].reshape(NF, 128, NM, 128).transpose(2, 1, 0, 3)
        ).astype(ml_dtypes.bfloat16)
        wr_dev = np.ascontiguousarray(
            Wr[e].reshape(NM, 128, NM, 128).transpose(2, 1, 0, 3)
        ).astype(ml_dtypes.bfloat16)
        in_maps.append({
            "xg": xg_dev, "xp": xp_dev, "maak": mk, "maar": mr,
            "wk": wk_dev, "wv": wv_dev, "wr": wr_dev,
        })

    import os
    trace = os.environ.get("MOE_KERNEL_TRACE", "0") == "1"
    kw = {}
    if trace:
        kw = {"trace": True, "trace_cores": [0]}
    res = run_bass_kernel_spmd(_get_nc(), in_maps,
                               core_ids=list(range(E)), **kw)
    last_run = res

    out_flat = np.zeros((S, M), np.float32)
    for e in range(E):
        outT = res.results[e]["outT"]            # [NC, 128, NM, CT]
        rows = outT.transpose(0, 3, 2, 1).reshape(CAP, M)
        out_flat[idx[e]] = rows[: len(idx[e])]
    out = out_flat.reshape(B, T, M)
    new_shift_state = np.ascontiguousarray(x[:, -1])
    return out, new_shift_state


# revision 6
# speedup vs baseline: 1.0046x; 1.0016x over previous
"""CMoE (hash-routed top-1 MoE + RWKV time-shift mix) on 8 TRN2 NeuronCores.

Strategy: expert-parallel. Core e owns expert e's weights. The hash
routing is a pure function of token_ids, so the dispatch/combine
(all-to-all in the sharding hint) is realized at input-sharding time: the
host computes each expert's kept-token list, gathers those tokens'
x/x-prev rows into feature-major per-core buffers, and scatters the
per-core outputs back into [B, T, m]. Everything with FLOPs — the
time-shift mix, all three matmuls, relu^2, sigmoid, and the r*kv
product — runs on-device.

Per-core device kernel (one expert, capacity 2048 tokens, 4 chunks of
512): activations live feature-major ([m-tile planes, tokens]) so every
matmul uses the natural weight layout as the stationary operand and
chains without any transposes. Matmuls run bf16 (inputs are rounded
on-chip by the DVE ops that produce them), accumulation fp32 in PSUM.
DMA is spread over three rings (sync: activations/outputs, scalar: Wk
stream, gpsimd: Wv/Wr) with explicit pacing deps so big transfers never
starve the critical path; the next chunk's loads + mix are
software-pipelined into the current chunk's second phase.
"""
from contextlib import ExitStack

import numpy as np
import ml_dtypes

import concourse.bacc as bacc
import concourse.tile as tile
import concourse.mybir as mybir
from concourse.tile import add_dep_helper
from concourse.bass_utils import run_bass_kernel_spmd

F32 = mybir.dt.float32
BF16 = mybir.dt.bfloat16
AF = mybir.ActivationFunctionType
AL = mybir.AluOpType

HASH_PRIME = 5099
B, T, M, F, E = 8, 2048, 1024, 2048, 8
S = B * T
CAP = max(4, -(-S // E))   # 2048
CT = 512
NM = M // 128              # 8
NF = F // 128              # 16
NC = CAP // CT             # 4


def _build():
    nc = bacc.Bacc(None)
    xg_d = nc.dram_tensor("xg", [NC, 128, NM, CT], BF16, kind="ExternalInput")
    xp_d = nc.dram_tensor("xp", [NC, 128, NM, CT], BF16, kind="ExternalInput")
    maak_d = nc.dram_tensor("maak", [128, NM], F32, kind="ExternalInput")
    maar_d = nc.dram_tensor("maar", [128, NM], F32, kind="ExternalInput")
    wk_d = nc.dram_tensor("wk", [NF, 128, NM, 128], BF16, kind="ExternalInput")
    wv_d = nc.dram_tensor("wv", [NM, 128, NF, 128], BF16, kind="ExternalInput")
    wr_d = nc.dram_tensor("wr", [NM, 128, NM, 128], BF16, kind="ExternalInput")
    out_d = nc.dram_tensor("outT", [NC, 128, NM, CT], F32, kind="ExternalOutput")

    with tile.TileContext(nc) as tc, ExitStack() as ctx:
        maa_p = ctx.enter_context(tc.tile_pool(name="maa", bufs=1))
        wr_p = ctx.enter_context(tc.tile_pool(name="wr", bufs=1))
        wk_p = ctx.enter_context(tc.tile_pool(name="wk", bufs=6))
        wv_p = ctx.enter_context(tc.tile_pool(name="wv", bufs=3))
        act_p = ctx.enter_context(tc.tile_pool(name="act", bufs=1))
        h_p = ctx.enter_context(tc.tile_pool(name="h", bufs=2))
        scr_p = ctx.enter_context(tc.tile_pool(name="scr", bufs=2))
        out_p = ctx.enter_context(tc.tile_pool(name="out", bufs=3))
        hps_p = ctx.enter_context(tc.tile_pool(name="hps", bufs=3, space="PSUM"))
        kvps_p = ctx.enter_context(tc.tile_pool(name="kvps", bufs=2, space="PSUM"))
        rps_p = ctx.enter_context(tc.tile_pool(name="rps", bufs=2, space="PSUM"))

        maak = maa_p.tile([128, NM], F32, name="maak_sb")
        maar = maa_p.tile([128, NM], F32, name="maar_sb")
        nc.sync.dma_start(maak[:], maak_d[:])
        nc.sync.dma_start(maar[:], maar_d[:])
        wr_sb = wr_p.tile([128, NM, NM, 128], BF16, name="wr_sb")

        def load_act_plane(ct, a, xg, xp):
            # alternate planes across the sync and gpsimd rings so arrivals
            # overlap instead of serializing on one FIFO; the scalar ring is
            # left to the wk stream (gpsimd's wr/wv loads are gated until
            # well after the fill window)
            eng = nc.sync if a % 2 == 0 else nc.gpsimd
            d1 = eng.dma_start(xg[:, a, :], xg_d[ct][:, a, :])
            d2 = eng.dma_start(xp[:, a, :], xp_d[ct][:, a, :])
            return [d1, d2]

        def mix_plane(xg, xp, xk, xr, a):
            dx = scr_p.tile([128, CT], F32, name="dx_sb", tag="dx")
            nc.vector.tensor_sub(dx[:], xp[:, a, :], xg[:, a, :])
            nc.vector.scalar_tensor_tensor(
                xk[:, a, :], dx[:], maak[:, a:a + 1], xg[:, a, :],
                op0=AL.mult, op1=AL.add)
            nc.vector.scalar_tensor_tensor(
                xr[:, a, :], dx[:], maar[:, a:a + 1], xg[:, a, :],
                op0=AL.mult, op1=AL.add)

        def new_act_tiles():
            xg = act_p.tile([128, NM, CT], BF16, name="xg_sb", tag="xg", bufs=1)
            xp = act_p.tile([128, NM, CT], BF16, name="xp_sb", tag="xp", bufs=1)
            xk = act_p.tile([128, NM, CT], BF16, name="xk_sb", tag="xk", bufs=2)
            xr = act_p.tile([128, NM, CT], BF16, name="xr_sb", tag="xr", bufs=2)
            return xg, xp, xk, xr

        # PE warmup: the HAM clock-gate holds the PE at 1.2GHz until it has
        # seen ~3.4us of sustained activity. Chunk 0's matmuls arrive in a
        # DMA-paced drip and would otherwise all run cold; burn ~8us of
        # zero matmuls first so real work starts at 2.4GHz.
        wpad = maa_p.tile([128, CT], BF16, name="warm_sb")
        nc.gpsimd.memset(wpad[:], 0)
        wps = hps_p.tile([128, CT], F32, name="warm_ps", tag="hps")
        for _ in range(30):
            nc.tensor.matmul(wps[:], wpad[:, :128], wpad[:],
                             start=True, stop=True, skip_group_check=True)

        # chunk 0 prologue: per-plane load + mix, so mm1 starts after the
        # first plane lands rather than after the full 4MB.
        xg, xp, xk, xr = new_act_tiles()
        for a in range(NM):
            load_act_plane(0, a, xg, xp)
            mix_plane(xg, xp, xk, xr, a)

        for ct in range(NC):
            # h^T[ft] = relu(Wk_ft^T @ xk)^2  -> h_sb bf16
            h_sb = h_p.tile([128, NF, CT], BF16, name="h_sb", tag="h")
            relus = []
            for ft in range(NF):
                wk_sb = wk_p.tile([128, NM, 128], BF16, name="wk_sb", tag="wk")
                nc.scalar.dma_start(wk_sb[:], wk_d[ft])
                hps = hps_p.tile([128, CT], F32, name="hps", tag="hps")
                for a in range(NM):
                    nc.tensor.matmul(hps[:], wk_sb[:, a, :], xk[:, a, :],
                                     start=(a == 0), stop=(a == NM - 1))
                hr = scr_p.tile([128, CT], F32, name="hr_sb", tag="hr")
                relus.append(nc.scalar.activation(hr[:], hps[:], AF.Relu))
                nc.vector.tensor_mul(h_sb[:, ft, :], hr[:], hr[:])

            if ct == 0:
                # wr's first consumer is the mt-loop below; 8 pieces paced
                # across mm1 so piece mt lands just before its first use.
                for wmt in range(NM):
                    wr_dma = nc.gpsimd.dma_start(wr_sb[:, wmt, :, :], wr_d[wmt])
                    add_dep_helper(wr_dma.ins, relus[min(2 + wmt, NF - 1)].ins,
                                   True, "pace wr piece behind mm1 progress")

            # software-pipelined next chunk: plane loads gated on mm1
            # progress; mix planes woven through the mt-loop below. All
            # planes must be in flight early enough that the woven mix never
            # blocks the in-order DVE stream mid-chunk.
            if ct + 1 < NC:
                xg_n, xp_n, xk_n, xr_n = new_act_tiles()
                for a in range(NM):
                    for d in load_act_plane(ct + 1, a, xg_n, xp_n):
                        add_dep_helper(d.ins, relus[min(a, NF - 1)].ins,
                                       True, "pace next-chunk act loads")
            else:
                xg_n = xp_n = xk_n = xr_n = None

            # kv^T[mt] = Wv_mt^T @ h ; r^T[mt] = sigmoid(Wr_mt^T @ xr)
            # out^T[mt] = r^T * kv^T
            for mt in range(NM):
                wv_sb = wv_p.tile([128, NF, 128], BF16, name="wv_sb", tag="wv")
                wv_dma = nc.gpsimd.dma_start(wv_sb[:], wv_d[mt])
                if ct == 0 and mt < 3:
                    add_dep_helper(wv_dma.ins, relus[7 + 2 * mt].ins, True,
                                   "pace early wv loads behind mm1 progress")
                kvps = kvps_p.tile([128, CT], F32, name="kvps", tag="kvps")
                for ftk in range(NF):
                    nc.tensor.matmul(kvps[:], wv_sb[:, ftk, :], h_sb[:, ftk, :],
                                     start=(ftk == 0), stop=(ftk == NF - 1))
                rps = rps_p.tile([128, CT], F32, name="rps", tag="rps")
                for kt in range(NM):
                    nc.tensor.matmul(rps[:], wr_sb[:, mt, kt, :], xr[:, kt, :],
                                     start=(kt == 0), stop=(kt == NM - 1))
                rsb = scr_p.tile([128, CT], F32, name="r_sb", tag="r")
                nc.scalar.activation(rsb[:], rps[:], AF.Sigmoid)
                ot = out_p.tile([128, CT], F32, name="ot_sb", tag="ot")
                nc.vector.tensor_mul(ot[:], rsb[:], kvps[:])
                nc.sync.dma_start(out_d[ct][:, mt, :], ot[:])
                if xg_n is not None:
                    mix_plane(xg_n, xp_n, xk_n, xr_n, mt)

            xg, xp, xk, xr = xg_n, xp_n, xk_n, xr_n

    nc.finalize()
    return nc


_NC_CACHE = None
last_run = None  # BassKernelResults of the most recent kernel() call


def _get_nc():
    global _NC_CACHE
    if _NC_CACHE is None:
        _NC_CACHE = _build()
    return _NC_CACHE


def _route(token_ids):
    """Reference-equivalent hash routing with capacity drop. Returns each
    expert's kept-token flat indices (arrival order) and their time-shift
    predecessor indices into [x_flat; shift_state]."""
    tok = token_ids.reshape(S).astype(np.int64)
    e_idx = (tok * HASH_PRIME) % E
    order = np.argsort(e_idx, kind="stable")
    counts = np.bincount(e_idx, minlength=E)
    starts = np.zeros(E, np.int64)
    starts[1:] = np.cumsum(counts)[:-1]
    prev = np.arange(S) - 1
    prev[np.arange(B) * T] = S + np.arange(B)   # row 0 reads shift_state
    idx, pidx = [], []
    for e in range(E):
        ke = order[starts[e]: starts[e] + min(counts[e], CAP)]
        idx.append(ke)
        pidx.append(prev[ke])
    return idx, pidx


def kernel(x, token_ids, shift_state, time_maa_k, time_maa_r, Wk, Wv, Wr):
    global last_run
    x = np.ascontiguousarray(np.asarray(x, dtype=np.float32))
    token_ids = np.asarray(token_ids)
    shift_state = np.asarray(shift_state, dtype=np.float32)
    Wk = np.asarray(Wk, dtype=np.float32)
    Wv = np.asarray(Wv, dtype=np.float32)
    Wr = np.asarray(Wr, dtype=np.float32)

    idx, pidx = _route(token_ids)

    x_flat = x.reshape(S, M)
    xcatT = np.ascontiguousarray(
        np.concatenate([x_flat, shift_state], axis=0).T)   # [M, S+B]
    mk = np.ascontiguousarray(
        np.asarray(time_maa_k, dtype=np.float32).reshape(M).reshape(8, 128).T)
    mr = np.ascontiguousarray(
        np.asarray(time_maa_r, dtype=np.float32).reshape(M).reshape(8, 128).T)

    in_maps = []
    for e in range(E):
        n_e = len(idx[e])
        xg = np.zeros((M, CAP), np.float32)
        xp = np.zeros((M, CAP), np.float32)
        xg[:, :n_e] = xcatT[:, idx[e]]
        xp[:, :n_e] = xcatT[:, pidx[e]]
        # [M, CAP] -> [NC, 128, NM, CT] chunk-major with m = a*128 + p, so a
        # chunk plane's load is fully contiguous per SBUF partition.
        xg_dev = np.ascontiguousarray(
            xg.reshape(NM, 128, NC, CT).transpose(2, 1, 0, 3)
        ).astype(ml_dtypes.bfloat16)
        xp_dev = np.ascontiguousarray(
            xp.reshape(NM, 128, NC, CT).transpose(2, 1, 0, 3)
        ).astype(ml_dtypes.bfloat16)
        wk_dev = np.ascontiguousarray(
            Wk[e].reshape(NM, 128, NF, 128).transpose(2, 1, 0, 3)
        ).astype(ml_dtypes.bfloat16)
        wv_dev = np.ascontiguousarray(
            Wv[e].reshape(NF, 128, NM, 128).transpose(2, 1, 0, 3)
        ).astype(ml_dtypes.bfloat16)
        wr_dev = np.ascontiguousarray(
            Wr[e].reshape(NM, 128, NM, 128).transpose(2, 1, 0, 3)
        ).astype(ml_dtypes.bfloat16)
        in_maps.append({
            "xg": xg_dev, "xp": xp_dev, "maak": mk, "maar": mr,
            "wk": wk_dev, "wv": wv_dev, "wr": wr_dev,
        })

    import os
    trace = os.environ.get("MOE_KERNEL_TRACE", "0") == "1"
    kw = {}
    if trace:
        kw = {"trace": True, "trace_cores": [0]}
    res = run_bass_kernel_spmd(_get_nc(), in_maps,
                               core_ids=list(range(E)), **kw)
    last_run = res

    out_flat = np.zeros((S, M), np.float32)
    for e in range(E):
        outT = res.results[e]["outT"]            # [NC, 128, NM, CT]
        rows = outT.transpose(0, 3, 2, 1).reshape(CAP, M)
        out_flat[idx[e]] = rows[: len(idx[e])]
    out = out_flat.reshape(B, T, M)
    new_shift_state = np.ascontiguousarray(x[:, -1])
    return out, new_shift_state


# revision 7
# speedup vs baseline: 1.0234x; 1.0188x over previous
"""CMoE (hash-routed top-1 MoE + RWKV time-shift mix) on 8 TRN2 NeuronCores.

Strategy: expert-parallel. Core e owns expert e's weights. The hash
routing is a pure function of token_ids, so the dispatch/combine
(all-to-all in the sharding hint) is realized at input-sharding time: the
host computes each expert's kept-token list, gathers those tokens'
x/x-prev rows into feature-major per-core buffers, and scatters the
per-core outputs back into [B, T, m]. Everything with FLOPs — the
time-shift mix, all three matmuls, relu^2, sigmoid, and the r*kv
product — runs on-device.

Per-core device kernel (one expert, capacity 2048 tokens, 4 chunks of
512): activations live feature-major ([m-tile planes, tokens]) so every
matmul uses the natural weight layout as the stationary operand and
chains without any transposes. Matmuls run bf16 (inputs are rounded
on-chip by the DVE ops that produce them), accumulation fp32 in PSUM.
DMA is spread over three rings (sync: activations/outputs, scalar: Wk
stream, gpsimd: Wv/Wr) with explicit pacing deps so big transfers never
starve the critical path; the next chunk's loads + mix are
software-pipelined into the current chunk's second phase.
"""
from contextlib import ExitStack

import numpy as np
import ml_dtypes

import concourse.bacc as bacc
import concourse.tile as tile
import concourse.mybir as mybir
from concourse.tile import add_dep_helper
from concourse.bass_utils import run_bass_kernel_spmd

F32 = mybir.dt.float32
BF16 = mybir.dt.bfloat16
AF = mybir.ActivationFunctionType
AL = mybir.AluOpType

HASH_PRIME = 5099
B, T, M, F, E = 8, 2048, 1024, 2048, 8
S = B * T
CAP = max(4, -(-S // E))   # 2048
CT = 512
NM = M // 128              # 8
NF = F // 128              # 16
NC = CAP // CT             # 4


def _build():
    nc = bacc.Bacc(None)
    xgp_d = nc.dram_tensor("xgp", [NC, 128, NM, 2, CT], BF16,
                           kind="ExternalInput")
    maak_d = nc.dram_tensor("maak", [128, NM], F32, kind="ExternalInput")
    maar_d = nc.dram_tensor("maar", [128, NM], F32, kind="ExternalInput")
    wk_d = nc.dram_tensor("wk", [NF, 128, NM, 128], BF16, kind="ExternalInput")
    wv_d = nc.dram_tensor("wv", [NM, 128, NF, 128], BF16, kind="ExternalInput")
    wr_d = nc.dram_tensor("wr", [NM, 128, NM, 128], BF16, kind="ExternalInput")
    out_d = nc.dram_tensor("outT", [NC, 128, NM, CT], F32, kind="ExternalOutput")

    with tile.TileContext(nc) as tc, ExitStack() as ctx:
        maa_p = ctx.enter_context(tc.tile_pool(name="maa", bufs=1))
        wr_p = ctx.enter_context(tc.tile_pool(name="wr", bufs=1))
        wk_p = ctx.enter_context(tc.tile_pool(name="wk", bufs=6))
        wv_p = ctx.enter_context(tc.tile_pool(name="wv", bufs=3))
        act_p = ctx.enter_context(tc.tile_pool(name="act", bufs=1))
        h_p = ctx.enter_context(tc.tile_pool(name="h", bufs=2))
        scr_p = ctx.enter_context(tc.tile_pool(name="scr", bufs=2))
        out_p = ctx.enter_context(tc.tile_pool(name="out", bufs=3))
        hps_p = ctx.enter_context(tc.tile_pool(name="hps", bufs=3, space="PSUM"))
        kvps_p = ctx.enter_context(tc.tile_pool(name="kvps", bufs=2, space="PSUM"))
        rps_p = ctx.enter_context(tc.tile_pool(name="rps", bufs=2, space="PSUM"))

        maak = maa_p.tile([128, NM], F32, name="maak_sb")
        maar = maa_p.tile([128, NM], F32, name="maar_sb")
        nc.sync.dma_start(maak[:], maak_d[:])
        nc.sync.dma_start(maar[:], maar_d[:])
        wr_sb = wr_p.tile([128, NM, NM, 128], BF16, name="wr_sb")

        def load_act_plane(ct, a, xgp):
            # one packed transfer per plane (xg and xp interleaved in DRAM)
            # halves the per-transfer fixed-cost count on the fill path;
            # alternate planes across the sync and gpsimd rings so arrivals
            # overlap instead of serializing on one FIFO (the scalar ring is
            # left to the wk stream; gpsimd's wr/wv loads are gated until
            # well after the fill window)
            eng = nc.sync if a % 2 == 0 else nc.gpsimd
            return [eng.dma_start(xgp[:, a, :, :], xgp_d[ct][:, a, :, :])]

        def mix_plane(xgp, xk, xr, a):
            xg_v = xgp[:, a, 0, :]
            xp_v = xgp[:, a, 1, :]
            dx = scr_p.tile([128, CT], F32, name="dx_sb", tag="dx")
            nc.vector.tensor_sub(dx[:], xp_v, xg_v)
            nc.vector.scalar_tensor_tensor(
                xk[:, a, :], dx[:], maak[:, a:a + 1], xg_v,
                op0=AL.mult, op1=AL.add)
            nc.vector.scalar_tensor_tensor(
                xr[:, a, :], dx[:], maar[:, a:a + 1], xg_v,
                op0=AL.mult, op1=AL.add)

        def new_act_tiles():
            xgp = act_p.tile([128, NM, 2, CT], BF16, name="xgp_sb", tag="xgp",
                             bufs=1)
            xk = act_p.tile([128, NM, CT], BF16, name="xk_sb", tag="xk", bufs=2)
            xr = act_p.tile([128, NM, CT], BF16, name="xr_sb", tag="xr", bufs=2)
            return xgp, xk, xr

        # PE warmup: the HAM clock-gate holds the PE at 1.2GHz until it has
        # seen ~3.4us of sustained activity. Chunk 0's matmuls arrive in a
        # DMA-paced drip and would otherwise all run cold; burn ~8us of
        # zero matmuls first so real work starts at 2.4GHz.
        wpad = maa_p.tile([128, CT], BF16, name="warm_sb")
        nc.gpsimd.memset(wpad[:], 0)
        wps = hps_p.tile([128, CT], F32, name="warm_ps", tag="hps")
        for _ in range(30):
            nc.tensor.matmul(wps[:], wpad[:, :128], wpad[:],
                             start=True, stop=True, skip_group_check=True)

        # chunk 0 prologue: per-plane load + mix, so mm1 starts after the
        # first plane lands rather than after the full 4MB.
        xgp, xk, xr = new_act_tiles()
        for a in range(NM):
            load_act_plane(0, a, xgp)
            mix_plane(xgp, xk, xr, a)

        for ct in range(NC):
            # h^T[ft] = relu(Wk_ft^T @ xk)^2  -> h_sb bf16
            h_sb = h_p.tile([128, NF, CT], BF16, name="h_sb", tag="h")
            relus = []
            for ft in range(NF):
                wk_sb = wk_p.tile([128, NM, 128], BF16, name="wk_sb", tag="wk")
                nc.scalar.dma_start(wk_sb[:], wk_d[ft])
                hps = hps_p.tile([128, CT], F32, name="hps", tag="hps")
                for a in range(NM):
                    nc.tensor.matmul(hps[:], wk_sb[:, a, :], xk[:, a, :],
                                     start=(a == 0), stop=(a == NM - 1))
                hr = scr_p.tile([128, CT], F32, name="hr_sb", tag="hr")
                relus.append(nc.scalar.activation(hr[:], hps[:], AF.Relu))
                nc.vector.tensor_mul(h_sb[:, ft, :], hr[:], hr[:])

            if ct == 0:
                # wr's first consumer is the mt-loop below; 8 pieces paced
                # across mm1 so piece mt lands just before its first use.
                for wmt in range(NM):
                    wr_dma = nc.gpsimd.dma_start(wr_sb[:, wmt, :, :], wr_d[wmt])
                    add_dep_helper(wr_dma.ins, relus[min(2 + wmt, NF - 1)].ins,
                                   True, "pace wr piece behind mm1 progress")

            # software-pipelined next chunk: plane loads gated on mm1
            # progress; mix planes woven through the mt-loop below. All
            # planes must be in flight early enough that the woven mix never
            # blocks the in-order DVE stream mid-chunk.
            if ct + 1 < NC:
                xgp_n, xk_n, xr_n = new_act_tiles()
                for a in range(NM):
                    for d in load_act_plane(ct + 1, a, xgp_n):
                        add_dep_helper(d.ins, relus[min(a, NF - 1)].ins,
                                       True, "pace next-chunk act loads")
            else:
                xgp_n = xk_n = xr_n = None

            # kv^T[mt] = Wv_mt^T @ h ; r^T[mt] = sigmoid(Wr_mt^T @ xr)
            # out^T[mt] = r^T * kv^T
            for mt in range(NM):
                wv_sb = wv_p.tile([128, NF, 128], BF16, name="wv_sb", tag="wv")
                wv_dma = nc.gpsimd.dma_start(wv_sb[:], wv_d[mt])
                if ct == 0 and mt < 3:
                    add_dep_helper(wv_dma.ins, relus[7 + 2 * mt].ins, True,
                                   "pace early wv loads behind mm1 progress")
                kvps = kvps_p.tile([128, CT], F32, name="kvps", tag="kvps")
                for ftk in range(NF):
                    nc.tensor.matmul(kvps[:], wv_sb[:, ftk, :], h_sb[:, ftk, :],
                                     start=(ftk == 0), stop=(ftk == NF - 1))
                rps = rps_p.tile([128, CT], F32, name="rps", tag="rps")
                for kt in range(NM):
                    nc.tensor.matmul(rps[:], wr_sb[:, mt, kt, :], xr[:, kt, :],
                                     start=(kt == 0), stop=(kt == NM - 1))
                rsb = scr_p.tile([128, CT], F32, name="r_sb", tag="r")
                nc.scalar.activation(rsb[:], rps[:], AF.Sigmoid)
                ot = out_p.tile([128, CT], F32, name="ot_sb", tag="ot")
                nc.vector.tensor_mul(ot[:], rsb[:], kvps[:])
                nc.sync.dma_start(out_d[ct][:, mt, :], ot[:])
                if xgp_n is not None:
                    mix_plane(xgp_n, xk_n, xr_n, mt)

            xgp, xk, xr = xgp_n, xk_n, xr_n

    nc.finalize()
    return nc


_NC_CACHE = None
last_run = None  # BassKernelResults of the most recent kernel() call


def _get_nc():
    global _NC_CACHE
    if _NC_CACHE is None:
        _NC_CACHE = _build()
    return _NC_CACHE


def _route(token_ids):
    """Reference-equivalent hash routing with capacity drop. Returns each
    expert's kept-token flat indices (arrival order) and their time-shift
    predecessor indices into [x_flat; shift_state]."""
    tok = token_ids.reshape(S).astype(np.int64)
    e_idx = (tok * HASH_PRIME) % E
    order = np.argsort(e_idx, kind="stable")
    counts = np.bincount(e_idx, minlength=E)
    starts = np.zeros(E, np.int64)
    starts[1:] = np.cumsum(counts)[:-1]
    prev = np.arange(S) - 1
    prev[np.arange(B) * T] = S + np.arange(B)   # row 0 reads shift_state
    idx, pidx = [], []
    for e in range(E):
        ke = order[starts[e]: starts[e] + min(counts[e], CAP)]
        idx.append(ke)
        pidx.append(prev[ke])
    return idx, pidx


def kernel(x, token_ids, shift_state, time_maa_k, time_maa_r, Wk, Wv, Wr):
    global last_run
    x = np.ascontiguousarray(np.asarray(x, dtype=np.float32))
    token_ids = np.asarray(token_ids)
    shift_state = np.asarray(shift_state, dtype=np.float32)
    Wk = np.asarray(Wk, dtype=np.float32)
    Wv = np.asarray(Wv, dtype=np.float32)
    Wr = np.asarray(Wr, dtype=np.float32)

    idx, pidx = _route(token_ids)

    x_flat = x.reshape(S, M)
    xcatT = np.ascontiguousarray(
        np.concatenate([x_flat, shift_state], axis=0).T)   # [M, S+B]
    mk = np.ascontiguousarray(
        np.asarray(time_maa_k, dtype=np.float32).reshape(M).reshape(8, 128).T)
    mr = np.ascontiguousarray(
        np.asarray(time_maa_r, dtype=np.float32).reshape(M).reshape(8, 128).T)

    in_maps = []
    for e in range(E):
        n_e = len(idx[e])
        xg = np.zeros((M, CAP), np.float32)
        xp = np.zeros((M, CAP), np.float32)
        xg[:, :n_e] = xcatT[:, idx[e]]
        xp[:, :n_e] = xcatT[:, pidx[e]]
        # [M, CAP] -> [NC, 128, NM, CT] chunk-major with m = a*128 + p, so a
        # chunk plane's load is fully contiguous per SBUF partition.
        packed = np.stack([xg.reshape(NM, 128, NC, CT),
                           xp.reshape(NM, 128, NC, CT)], axis=3)
        xgp_dev = np.ascontiguousarray(
            packed.transpose(2, 1, 0, 3, 4)).astype(ml_dtypes.bfloat16)
        wk_dev = np.ascontiguousarray(
            Wk[e].reshape(NM, 128, NF, 128).transpose(2, 1, 0, 3)
        ).astype(ml_dtypes.bfloat16)
        wv_dev = np.ascontiguousarray(
            Wv[e].reshape(NF, 128, NM, 128).transpose(2, 1, 0, 3)
        ).astype(ml_dtypes.bfloat16)
        wr_dev = np.ascontiguousarray(
            Wr[e].reshape(NM, 128, NM, 128).transpose(2, 1, 0, 3)
        ).astype(ml_dtypes.bfloat16)
        in_maps.append({
            "xgp": xgp_dev, "maak": mk, "maar": mr,
            "wk": wk_dev, "wv": wv_dev, "wr": wr_dev,
        })

    import os
    trace = os.environ.get("MOE_KERNEL_TRACE", "0") == "1"
    kw = {}
    if trace:
        kw = {"trace": True, "trace_cores": [0]}
    res = run_bass_kernel_spmd(_get_nc(), in_maps,
                               core_ids=list(range(E)), **kw)
    last_run = res

    out_flat = np.zeros((S, M), np.float32)
    for e in range(E):
        outT = res.results[e]["outT"]            # [NC, 128, NM, CT]
        rows = outT.transpose(0, 3, 2, 1).reshape(CAP, M)
        out_flat[idx[e]] = rows[: len(idx[e])]
    out = out_flat.reshape(B, T, M)
    new_shift_state = np.ascontiguousarray(x[:, -1])
    return out, new_shift_state
